# revision 72
# baseline (speedup 1.0000x reference)
"""Trainium2 Bass kernel for nn_Attention_35021163332119.

Full multi-head attention: qkv = x @ w_qkv; RoPE(q, k); softmax(q k^T / sqrt(dh)) v;
out = heads @ w_out + b_out.  B=2, N=2048, DIM=1024, H=16, DH=64.

Sharding: 8 cores = (batch b in {0,1}) x (head-group g in {0..3} of 4 heads).
Each core computes its 4 heads end-to-end plus the partial output projection
for its head-group's rows of w_out; the host sums the 4 partials per batch
(bf16 partials, fp32 accumulation) and adds b_out.

Schedule: the kernel is PE-paced (~190 us of MATMUL busy at 2.4 GHz), so
the whole schedule keeps the Tensor engine dense and its HAM clock hot
(any idle re-throttles it to 1.2 GHz): deadline-ordered fused input DMAs
(a dma_start costs ~0.6 us of sequencer issue time, so per-k transfers
are fused into 3D-AP group loads), a per-half staged first block whose
scores start as soon as {wqk, xT half 0, trig} land, warm p2t spins
filling every DMA/rope-latency hole, and all remaining PE work (QKV
pair 1, RoPE, output projection) drip-fed as per-jj "filler" pieces.
The tail is latency-lean: the final block's normalization broadcasts via
GpSimd + partition-parallel reciprocal, trailing projection evacuations
run on the (idle-by-then) Scalar engine, and output DMAs issue from
three different sequencers.

On-core layout: x is host-transposed to xT [DIM, N]; q,k are produced
transposed ([dh, n], head pairs stacked on 128 partitions); v is produced
in natural [n, dh] layout with an extra ones column so the PV matmul (M=65)
also accumulates the softmax denominator in row 64.  RoPE's interleaved
pair-rotation is a 128x128 +/-1 permutation matmul on the PE plus DVE
multiplies against cos/sin tables (the cos one on GpSimd in the startup
phase).  PSUM->SBUF copies run on Scalar only while the exp stream has
not started; everything later uses DVE.
"""

import numpy as np

B, N, DIM, H, DH = 2, 2048, 1024, 16, 64
ROPE_BASE = 10000.0
SCALE = DH ** -0.5
N_CORES = 8
G = 4                 # heads per core
KT = DIM // 128       # contraction tiles
NT = N // 128         # sequence tiles

_cache = {}


def _rope_tables():
    inv_freq = (1.0 / (ROPE_BASE ** (np.arange(0, DH, 2, dtype=np.float32) / DH)))
    t = np.arange(N, dtype=np.float32)
    freqs = t[:, None] * inv_freq[None, :]          # [N, DH/2]
    freqs = np.repeat(freqs, 2, axis=-1)            # [N, DH] interleaved
    cosT = np.cos(freqs).T.astype(np.float32)       # [DH, N]
    sinT = np.sin(freqs).T.astype(np.float32)
    cos2 = np.concatenate([cosT, cosT], axis=0)     # [128, N] two heads stacked
    sin2 = np.concatenate([sinT, sinT], axis=0)
    return np.ascontiguousarray(cos2), np.ascontiguousarray(sin2)


def _p2t():
    # rot = P2 @ qT with P2 = blockdiag(P, P), P[2t, 2t+1] = -1, P[2t+1, 2t] = 1
    # matmul computes lhsT.T @ rhs, so pass P2.T
    p = np.zeros((DH, DH), dtype=np.float32)
    for t in range(DH // 2):
        p[2 * t, 2 * t + 1] = -1.0
        p[2 * t + 1, 2 * t] = 1.0
    p2 = np.zeros((128, 128), dtype=np.float32)
    p2[:DH, :DH] = p
    p2[DH:, DH:] = p
    return np.ascontiguousarray(p2.T)


def _build():
    if "nc" in _cache:
        return _cache["nc"]

    import concourse.mybir as mybir
    import concourse.tile as tile
    from concourse import bacc

    F32 = mybir.dt.float32
    F32R = mybir.dt.float32r
    BF16 = mybir.dt.bfloat16
    EXP = mybir.ActivationFunctionType.Exp

    nc = bacc.Bacc("TRN2", target_bir_lowering=False, debug=False)
    xT_d = nc.dram_tensor("xT", [DIM, N], BF16, kind="ExternalInput")
    wqk_d = nc.dram_tensor("wqk", [DIM, 4 * 128], BF16, kind="ExternalInput")
    wv_d = nc.dram_tensor("wv", [DIM, G * DH], BF16, kind="ExternalInput")
    wout_d = nc.dram_tensor("wout", [G * DH, DIM], BF16, kind="ExternalInput")
    cos_d = nc.dram_tensor("cos2", [128, N], BF16, kind="ExternalInput")
    sin_d = nc.dram_tensor("sin2", [128, N], BF16, kind="ExternalInput")
    p2t_d = nc.dram_tensor("p2t", [128, 128], BF16, kind="ExternalInput")
    part_d = nc.dram_tensor("part", [N, DIM], BF16, kind="ExternalOutput")

    with tile.TileContext(nc) as tc:
        with tc.tile_pool(name="persist", bufs=1) as persist, \
             tc.tile_pool(name="att", bufs=8) as att, \
             tc.tile_pool(name="norm_w", bufs=2) as norm_w, \
             tc.tile_pool(name="tailw", bufs=1) as tailw, \
             tc.tile_pool(name="outp", bufs=3) as outp, \
             tc.tile_pool(name="xph", bufs=1) as xph, \
             tc.tile_pool(name="rope_w", bufs=2) as rope_w, \
             tc.tile_pool(name="stash", bufs=4) as stash, \
             tc.tile_pool(name="ps", bufs=3, space="PSUM") as ps, \
             tc.tile_pool(name="pso", bufs=2, space="PSUM") as pso:

            # ---- persistent tiles ----
            # bf16 q/k: enables PE fast-weight-load on the scores matmuls
            # (halves the exposed LDWEIGHTS between row-group pairs) and 2x
            # DVE modes on the rope elementwise ops
            qk_sb = [persist.tile([128, N], BF16, tag=f"qk{m}", name=f"qk{m}")
                     for m in range(4)]          # q01T, q23T, k01T, k23T
            v_aug = persist.tile([128, NT, G, DH + 1], BF16, tag="vaug")
            wout_sb = [persist.tile([128, DIM], BF16, tag=f"wo{kk}", name=f"wo{kk}")
                       for kk in range(2)]
            outT = [persist.tile([128, N], BF16, tag=f"outT{p}", name=f"outT{p}")
                    for p in range(2)]

            # ---- phase-1 tiles (fused along the k-tile dim so one
            # dma_start loads a whole half-column group: a dma_start costs
            # ~0.6 us of sequencer issue time, so per-k transfers would make
            # the input phase issue-rate-bound) ----
            xT_all = xph.tile([128, KT, N], BF16, tag="xTall", name="xTall")
            wqk_all = xph.tile([128, KT, 4 * 128], BF16, tag="wqkall",
                               name="wqkall")
            wv_all = xph.tile([128, KT, G * DH], BF16, tag="wvall",
                              name="wvall")
            cos2 = xph.tile([128, N], BF16, tag="cos2")
            sin2 = xph.tile([128, N], BF16, tag="sin2")
            p2t = xph.tile([128, 128], BF16, tag="p2t")
            ones_bc = xph.tile([128, DH], F32, tag="ones_bc")
            warm = xph.tile([128, 8], F32, tag="warm")
            nc.vector.memset(ones_bc, 1.0)

            # preload the exp table set on the Scalar engine during DMA wait
            nc.vector.memset(warm, 0.0)
            nc.scalar.activation(warm, warm, EXP, scale=1.0)

            # ---- input DMAs, deadline-ordered, one fused start per group ----
            xT_src = xT_d.ap().rearrange("(t p) n -> p t n", p=128)
            wqk_src = wqk_d.ap().rearrange("(t p) m -> p t m", p=128)
            # p2t first: the PE warm-up matmuls below spin on it during the
            # DMA wait so the HAM clock gate opens before real work arrives
            nc.sync.dma_start(out=p2t, in_=p2t_d.ap())
            # q01/k01 weight columns + xT half 0 in fine-grained pieces so
            # the first chain's matmuls start as each piece lands
            nc.sync.dma_start(out=wqk_all[:, :, 256:384],
                              in_=wqk_src[:, :, 256:384])
            nc.sync.dma_start(out=wqk_all[:, :, 0:128],
                              in_=wqk_src[:, :, 0:128])
            nc.sync.dma_start(out=xT_all[:, 0:4, 0:512],
                              in_=xT_src[:, 0:4, 0:512])
            nc.sync.dma_start(out=xT_all[:, 4:KT, 0:512],
                              in_=xT_src[:, 4:KT, 0:512])
            nc.sync.dma_start(out=cos2[:, 0:1024], in_=cos_d.ap()[:, 0:1024])
            nc.sync.dma_start(out=sin2[:, 0:1024], in_=sin_d.ap()[:, 0:1024])
            nc.sync.dma_start(out=wqk_all[:, :, 128:256],
                              in_=wqk_src[:, :, 128:256])
            nc.sync.dma_start(out=wqk_all[:, :, 384:512],
                              in_=wqk_src[:, :, 384:512])
            nc.sync.dma_start(out=xT_all[:, :, 512:1024],
                              in_=xT_src[:, :, 512:1024])
            nc.sync.dma_start(
                out=wv_all,
                in_=wv_d.ap().rearrange("(t p) m -> p t m", p=128))
            nc.sync.dma_start(out=cos2[:, 1024:2048],
                              in_=cos_d.ap()[:, 1024:2048])
            nc.sync.dma_start(out=sin2[:, 1024:2048],
                              in_=sin_d.ap()[:, 1024:2048])
            nc.sync.dma_start(out=xT_all[:, :, 1024:1536],
                              in_=xT_src[:, :, 1024:1536])
            nc.sync.dma_start(out=xT_all[:, :, 1536:2048],
                              in_=xT_src[:, :, 1536:2048])
            for kk in range(2):
                nc.sync.dma_start(
                    out=wout_sb[kk],
                    in_=wout_d.ap().rearrange("(t p) m -> t p m", p=128)[kk])
            nc.vector.memset(v_aug[:, :, :, DH:DH + 1], 1.0)

            # ---- building blocks ----
            def qk_half(m, c2, half, slot, pair_ps, on_act):
                """8 accumulating matmuls for the 512-wide half (m, c2,
                half), written into `slot` (0/1) of a shared [128,1024] PSUM
                tile (two independent bank-aligned accumulation groups),
                then copied out to qk_sb on Scalar (pre-exp-stream) or DVE."""
                csl = slice(c2 * 1024 + half * 512, c2 * 1024 + (half + 1) * 512)
                sl = slice(slot * 512, (slot + 1) * 512)
                for k in range(KT):
                    nc.tensor.matmul(
                        pair_ps[:, sl],
                        wqk_all[:, k, m * 128:(m + 1) * 128],
                        xT_all[:, k, csl],
                        start=(k == 0), stop=(k == KT - 1))
                if on_act:
                    nc.scalar.copy(qk_sb[m][:, csl], pair_ps[:, sl])
                else:
                    nc.vector.tensor_copy(qk_sb[m][:, csl], pair_ps[:, sl])

            def rope_m_half(m, c2, half, cos_on_gpsimd=False):
                csl = slice(c2 * 1024 + half * 512, c2 * 1024 + (half + 1) * 512)
                rot_ps = ps.tile([128, 1024], F32, tag="s", name="mm_rot")
                nc.tensor.matmul(rot_ps[:, 0:512], p2t, qk_sb[m][:, csl],
                                 start=True, stop=True)
                tmp = rope_w.tile([128, 1024], BF16, tag="ropetmp")
                nc.vector.tensor_mul(tmp[:, 0:512], rot_ps[:, 0:512],
                                     sin2[:, csl])
                # the cos multiply touches only SBUF, so in the (gpsimd-idle)
                # startup phase it can run in parallel with the sin multiply
                cos_eng = nc.gpsimd if cos_on_gpsimd else nc.vector
                cos_eng.tensor_mul(qk_sb[m][:, csl], qk_sb[m][:, csl],
                                   cos2[:, csl])
                nc.vector.tensor_add(qk_sb[m][:, csl], qk_sb[m][:, csl],
                                     tmp[:, 0:512])

            def v_pair(t2):
                """v for sequence tiles 2*t2, 2*t2+1 in one PSUM tile."""
                mm_ps = ps.tile([128, 1024], F32, tag="s", name="mm_v")
                for sub in range(2):
                    tn = 2 * t2 + sub
                    for k in range(KT):
                        nc.tensor.matmul(
                            mm_ps[:, sub * 256:(sub + 1) * 256],
                            xT_all[:, k, tn * 128:(tn + 1) * 128],
                            wv_all[:, k, :],
                            start=(k == 0), stop=(k == KT - 1))
                nc.vector.tensor_copy(
                    v_aug[:, 2 * t2:2 * t2 + 2, :, 0:DH],
                    mm_ps[:, 0:512].rearrange("p (t h d) -> p t h d", t=2, h=G))

            # ---- attention machinery ----
            def att_begin():
                return {"o_ps": [pso.tile([DH + 1, 512], F32, tag="o",
                                          name=f"o{hh}") for hh in range(2)],
                        "pends": []}

            def emit_pv(p, st, jj, exps):
                for hh in range(2):
                    for half in range(2):
                        j = 2 * jj + half
                        nc.tensor.matmul(
                            st["o_ps"][hh],
                            v_aug[:, j, 2 * p + hh, :],
                            exps[hh][:, half * 512:(half + 1) * 512],
                            start=(j == 0), stop=(j == NT - 1))

            def att_jj(p, iq, jj, st, fill=None):
                # fill BEFORE the scores: fill tiles rotate the same 3-buf
                # PSUM pool as s_ps, and in this order the buffer a scores
                # alloc recycles was last consumed by a fast DVE copy (the
                # fill's) rather than by the previous jj's slow exp — that
                # one jj of rotation slack removes a per-jj scores stall
                if fill is not None:
                    fill()
                qT, kTt = qk_sb[p], qk_sb[2 + p]
                isl = slice(iq * 512, (iq + 1) * 512)
                s_ps = [ps.tile([128, 1024], F32, tag="s", name=f"s{hh}")
                        for hh in range(2)]
                for half in range(2):
                    j = 2 * jj + half
                    jsl = slice(j * 128, (j + 1) * 128)
                    for hh in range(2):
                        hsl = slice(hh * DH, (hh + 1) * DH)
                        nc.tensor.matmul(
                            s_ps[hh][:, half * 512:(half + 1) * 512],
                            kTt[hsl, jsl], qT[hsl, isl],
                            start=True, stop=True)
                exps = []
                for hh in range(2):
                    expT = att.tile([128, 1024], BF16, tag="exp")
                    nc.scalar.activation(expT, s_ps[hh], EXP, scale=SCALE)
                    exps.append(expT)
                # depth-2 PV pipeline: a PV group is emitted two jj after its
                # exps, so it never waits on the second head's activation
                st["pends"].append((jj, exps))
                if len(st["pends"]) > 2:
                    j0, e0 = st["pends"].pop(0)
                    emit_pv(p, st, j0, e0)

            def att_end(p, iq, st, fast_tail=False):
                for (j0, e0) in st["pends"]:
                    emit_pv(p, st, j0, e0)
                st["pends"] = []
                isl = slice(iq * 512, (iq + 1) * 512)
                # hh=1 first: its chain ends in an SBUF->SBUF DMA hop into
                # outT, which then overlaps hh=0's direct DVE write -- the
                # projection consumers wait on whichever finishes last
                for hh in (1, 0):
                    o_sb = norm_w.tile([DH + 1, 512], F32, tag=f"osb{hh}",
                                       name=f"osb{hh}")
                    nc.vector.tensor_copy(o_sb, st["o_ps"][hh])
                    if fast_tail:
                        # latency-lean: reciprocal in place on partition 64,
                        # broadcast via a K=1 matmul — no DMA hop, no gpsimd
                        nc.vector.reciprocal_approx_fast(
                            o_sb[DH:DH + 1, :], o_sb[DH:DH + 1, :])
                        bc_ps = ps.tile([128, 1024], F32, tag="s", name="bc")
                        nc.tensor.matmul(bc_ps[0:DH, 0:512],
                                         ones_bc[DH:DH + 1, 0:DH],
                                         o_sb[DH:DH + 1, :],
                                         start=True, stop=True)
                        bc = bc_ps[0:DH, 0:512]
                    else:
                        recip0 = norm_w.tile([1, 512], F32, tag=f"r0{hh}",
                                             name=f"r0{hh}")
                        nc.sync.dma_start(out=recip0, in_=o_sb[DH:DH + 1, :])
                        nc.vector.reciprocal_approx_fast(recip0, recip0)
                        bc = norm_w.tile([DH, 512], F32, tag=f"bc{hh}",
                                         name=f"bc{hh}")
                        nc.gpsimd.partition_broadcast(bc, recip0)
                    if hh == 0:
                        nc.vector.tensor_mul(outT[p][0:DH, isl],
                                             o_sb[0:DH, :], bc)
                    else:
                        tmpb = norm_w.tile([DH, 512], BF16, tag="tmpb")
                        nc.vector.tensor_mul(tmpb, o_sb[0:DH, :], bc)
                        nc.sync.dma_start(out=outT[p][DH:2 * DH, isl],
                                          in_=tmpb)

            def att_norm_tail_recips(st):
                """Phase 1 of the final block's latency-lean norm: broadcast
                each head's denominator row straight out of PSUM on the
                (idle) GpSimd engine, then take the reciprocal partition-
                parallel on DVE.  No DMA hop, no PE broadcast matmul."""
                recs = []
                for hh in (1, 0):
                    dsb = tailw.tile([1, 512], F32, tag=f"dt{hh}",
                                     name=f"dt{hh}")
                    nc.vector.tensor_copy(dsb, st["o_ps"][hh][DH:DH + 1, :])
                    bc = tailw.tile([DH, 512], F32, tag=f"bt{hh}",
                                    name=f"bt{hh}")
                    nc.gpsimd.partition_broadcast(bc, dsb)
                    nc.vector.reciprocal_approx_fast(bc, bc)
                    recs.append((hh, bc))
                return recs

            def att_norm_tail_muls(p, iq, st, recs):
                """Phase 2: scale the raw PSUM outputs by the broadcast
                reciprocals and write both head halves straight to outT."""
                isl = slice(iq * 512, (iq + 1) * 512)
                for hh, bc in recs:
                    nc.vector.tensor_mul(
                        outT[p][hh * DH:(hh + 1) * DH, isl],
                        st["o_ps"][hh][0:DH, :], bc)

            tail_mode = {"on": False}

            def proj_half(tn, c2, holder):
                """One 512-wide half of projection tile tn; copy+DMA on the
                second half.  In tail mode the PSUM evacuation runs on the
                (by then idle) Scalar engine so DVE is free for the final
                norm muls and adds."""
                if c2 == 0:
                    holder.clear()
                    holder.append(ps.tile([128, 1024], F32, tag="s",
                                          name="f_ps"))
                f_ps = holder[0]
                nsl = slice(tn * 128, (tn + 1) * 128)
                c2sl = slice(c2 * 512, (c2 + 1) * 512)
                for kk in range(2):
                    nc.tensor.matmul(
                        f_ps[:, c2sl],
                        outT[kk][:, nsl], wout_sb[kk][:, c2sl],
                        start=(kk == 0), stop=(kk == 1))
                if c2 == 1:
                    out_sb = outp.tile([128, DIM], BF16, tag="osb")
                    if tail_mode["on"]:
                        nc.scalar.copy(out_sb, f_ps)
                    else:
                        nc.vector.tensor_copy(out_sb, f_ps)
                    pdram = part_d.ap().rearrange("(t p) m -> t p m", p=128)[tn]
                    for phh in range(2):
                        psl2 = slice(phh * 64, (phh + 1) * 64)
                        nc.sync.dma_start(out=pdram[psl2, :],
                                          in_=out_sb[psl2, :])

            def proj_tile(tn):
                holder = []
                for c2 in range(2):
                    proj_half(tn, c2, holder)

            # ---- emission ----
            # Scheduling principle: the Tensor engine must stay ~100% dense
            # (its HAM clock gate re-throttles to 1.2 GHz if it idles and
            # never re-warms without a ~3.4us sustained-busy window), while
            # the Scalar engine's exp stream should pace the kernel.  So
            # every attention jj carries a small "filler" slice of the
            # non-attention PE work, slightly oversubscribing the PE.

            # PE warm-up: tiny matmuls on p2t spin during the input DMA
            # wait so the HAM opens the clock gate before the QKV work lands.
            # warm_fill() is also sprinkled into otherwise-idle early slots:
            # any PE idle resets the clock ramp back to 1.2 GHz, which would
            # make the whole DMA-gated prefix run at half speed.
            warm_ps = ps.tile([128, 1024], F32, tag="s", name="warm_ps")

            def warm_fill(n):
                for i in range(n):
                    nc.tensor.matmul(warm_ps[:, 0:128], p2t, p2t,
                                     start=(i == 0), stop=(i == n - 1))

            warm_fill(20)

            # First block (pair 0, i-quarter 0), staged per 512-wide half so
            # the first scores fire as soon as {wqk, xT half 0, trig chunk 0}
            # land (~17 us) instead of waiting for the full chunk-0 QKV.
            # Block (0,0) only needs q01 c0h0 (i 0:512) plus k01 halves
            # progressively (jj0-1: c0h0, jj2-3: c0h1, jj4-5: c1h0,
            # jj6-7: c1h1); each k01 half-chain is paired in its PSUM tile
            # with a pair-1 half-chain (q23/k23/q01-next) whose rope is
            # deferred, and the v chains ride as per-jj fills.
            st = att_begin()
            pA = ps.tile([128, 1024], F32, tag="s", name="chA")
            qk_half(2, 0, 0, 0, pA, True)
            qk_half(0, 0, 0, 1, pA, True)
            rope_m_half(2, 0, 0, cos_on_gpsimd=True)
            rope_m_half(0, 0, 0, cos_on_gpsimd=True)
            warm_fill(20)      # spin out the rope DVE latency at full clock
            att_jj(0, 0, 0, st, fill=(lambda: warm_fill(10)))
            att_jj(0, 0, 1, st, fill=(lambda: warm_fill(10)))
            pB2 = ps.tile([128, 1024], F32, tag="s", name="chB")
            qk_half(2, 0, 1, 0, pB2, False)
            qk_half(1, 0, 0, 1, pB2, False)
            rope_m_half(2, 0, 1, cos_on_gpsimd=True)
            warm_fill(8)
            att_jj(0, 0, 2, st, fill=(lambda: v_pair(0)))
            att_jj(0, 0, 3, st, fill=(lambda: v_pair(1)))
            pC = ps.tile([128, 1024], F32, tag="s", name="chC")
            qk_half(2, 1, 0, 0, pC, False)
            qk_half(3, 1, 0, 1, pC, False)
            rope_m_half(2, 1, 0, cos_on_gpsimd=True)
            warm_fill(8)
            att_jj(0, 0, 4, st, fill=(lambda: (v_pair(2), v_pair(3))))
            att_jj(0, 0, 5, st, fill=(lambda: v_pair(4)))
            pD = ps.tile([128, 1024], F32, tag="s", name="chD")
            qk_half(2, 1, 1, 0, pD, False)
            qk_half(0, 0, 1, 1, pD, False)
            rope_m_half(2, 1, 1)
            warm_fill(8)
            att_jj(0, 0, 6, st, fill=(lambda: (v_pair(5), v_pair(6))))
            # q01 c0h1 (computed in chain D) feeds block (0,1)'s very first
            # scores: rope it inside jj7 so its DVE work clears during this
            # block's tail instead of stalling the next block's start
            att_jj(0, 0, 7, st, fill=(lambda: (v_pair(7),
                                               rope_m_half(0, 0, 1))))
            att_end(0, 0, st)

            # filler pieces: remaining QKV half-chains (kpair granularity)
            # + their ropes, in deadline order; consumed up to 2 per jj
            # across blocks (0,1)-(0,3) and the spare slots of (1,0)
            fillers = []

            def qk_kpair_h(m, c2, half, kp, holder):
                csl = slice(c2 * 1024 + half * 512,
                            c2 * 1024 + (half + 1) * 512)
                if kp == 0:
                    holder.clear()
                    holder.append(ps.tile([128, 1024], F32, tag="s",
                                          name="qk1"))
                mm_ps = holder[0]
                for k in range(2 * kp, 2 * kp + 2):
                    nc.tensor.matmul(
                        mm_ps[:, 0:512],
                        wqk_all[:, k, m * 128:(m + 1) * 128],
                        xT_all[:, k, csl],
                        start=(k == 0), stop=(k == KT - 1))
                if kp == 3:
                    nc.vector.tensor_copy(qk_sb[m][:, csl], mm_ps[:, 0:512])

            def add_chain(m, c2, half):
                h2 = []
                for kp in range(4):
                    fillers.append(
                        lambda m=m, c2=c2, half=half, kp=kp, h2=h2:
                        qk_kpair_h(m, c2, half, kp, h2))
                fillers.append(
                    lambda m=m, c2=c2, half=half: rope_m_half(m, c2, half))

            add_chain(0, 1, 0)                            # q01 c1h0: (0,2)
            add_chain(3, 0, 0)                            # k23 c0h0: (1,0)
            add_chain(0, 1, 1)                            # q01 c1h1: (0,3)
            add_chain(3, 0, 1)                            # k23 c0h1: (1,0)
            add_chain(3, 1, 1)                            # k23 c1h1: (1,0)
            fillers.append(lambda: rope_m_half(3, 1, 0))  # chain in (0,0)
            fillers.append(lambda: rope_m_half(1, 0, 0))  # chain in (0,0)
            add_chain(1, 0, 1)                            # q23 c0h1: (1,1)
            add_chain(1, 1, 0)                            # q23 c1h0: (1,2)
            add_chain(1, 1, 1)                            # q23 c1h1: (1,3)

            for (p, iq) in [(0, 1), (0, 2), (0, 3)]:
                st = att_begin()
                for jj in range(NT // 2):
                    fill = None
                    if jj >= 1 and fillers:
                        pieces = [fillers.pop(0)]
                        if fillers:
                            pieces.append(fillers.pop(0))
                        fill = (lambda ps_=pieces: [f() for f in ps_])
                    att_jj(p, iq, jj, st, fill)
                att_end(p, iq, st)

            # two-phase projection for the trailing tiles 12-15: the
            # outT[0]-side accumulation only needs pair-0 results, so it
            # runs as fills inside block (1, 0); after the last block only
            # the outT[1]-side matmuls + add + DMA remain.
            pstash = {}

            def projA(tn):
                nsl = slice(tn * 128, (tn + 1) * 128)
                f_ps = ps.tile([128, 1024], F32, tag="s", name="pA")
                for c2 in range(2):
                    c2sl = slice(c2 * 512, (c2 + 1) * 512)
                    nc.tensor.matmul(f_ps[:, c2sl], outT[0][:, nsl],
                                     wout_sb[0][:, c2sl],
                                     start=True, stop=True)
                sb = stash.tile([128, DIM], F32, tag="pst", name=f"pst{tn}")
                nc.vector.tensor_copy(sb, f_ps)
                pstash[tn] = sb

            def projB(tn):
                """Tail projection: DVE add folds in the stashed outT0-side
                partial; the 4 row-contiguous 32 KB output chunks issue from
                Scalar+Sync (GpSimd must not issue the final transfers — its
                exit drain waits on its own queue and delays the finale)."""
                nsl = slice(tn * 128, (tn + 1) * 128)
                f_ps = ps.tile([128, 1024], F32, tag="s", name="pB")
                for c2 in range(2):
                    c2sl = slice(c2 * 512, (c2 + 1) * 512)
                    nc.tensor.matmul(f_ps[:, c2sl], outT[1][:, nsl],
                                     wout_sb[1][:, c2sl],
                                     start=True, stop=True)
                out_sb = outp.tile([128, DIM], BF16, tag="osb")
                nc.vector.tensor_add(out_sb, f_ps, pstash[tn])
                pdram = part_d.ap().rearrange("(t p) m -> t p m", p=128)[tn]
                engs = (nc.scalar, nc.sync, nc.scalar, nc.sync)
                for phh, eng in enumerate(engs):
                    psl2 = slice(phh * 32, (phh + 1) * 32)
                    eng.dma_start(out=pdram[psl2, :], in_=out_sb[psl2, :])

            st = att_begin()
            for jj in range(NT // 2):
                fill = None
                if 2 <= jj < 6:
                    fill = (lambda tn=10 + jj: projA(tn))
                elif jj >= 1 and fillers:
                    pieces = [fillers.pop(0)]
                    if fillers:
                        pieces.append(fillers.pop(0))
                    fill = (lambda ps_=pieces: [f() for f in ps_])
                att_jj(1, 0, jj, st, fill)
            att_end(1, 0, st)
            # anything q23 still owes must precede block (1,1)'s scores
            while fillers:
                fillers.pop(0)()

            # projection fillers for tiles 0-11: tile tn needs outT i-chunk
            # tn//4, finished at att_end(1, tn//4); two halves per tile, one
            # per jj starting two jj into the following block
            projq = []
            for (bi, (p, iq)) in enumerate([(1, 1), (1, 2), (1, 3)]):
                for tn in range(4 * bi, 4 * bi + 4):
                    h3 = []
                    for c2 in range(2):
                        projq.append(
                            (bi, lambda tn=tn, c2=c2, h3=h3:
                             proj_half(tn, c2, h3)))
                st = att_begin()
                for jj in range(NT // 2):
                    fill = None
                    if jj >= 2 and projq and projq[0][0] <= bi:
                        fill = projq.pop(0)[1]
                        if jj >= 4 and projq and projq[0][0] < bi:
                            f1, f2 = fill, projq.pop(0)[1]
                            fill = (lambda a=f1, b=f2: (a(), b()))
                    att_jj(p, iq, jj, st, fill)
                if (p, iq) != (1, 3):
                    att_end(p, iq, st)
                else:
                    # final block: drain PV only, then put all remaining
                    # independent PE work BEFORE the norm chain so the
                    # in-order PE queue isn't blocked behind the DVE recips
                    for (j0, e0) in st["pends"]:
                        emit_pv(p, st, j0, e0)
                    st["pends"] = []
            recs = att_norm_tail_recips(st)
            # muls BEFORE the projq drain so they aren't queued behind the
            # leftover projection evacuations on DVE
            att_norm_tail_muls(1, 3, st, recs)
            tail_mode["on"] = True
            for (_, fn) in projq:
                fn()
            # keep the PE clock warm through the final norm chain so the
            # trailing projection matmuls run at 2.4 GHz
            warm2_ps = ps.tile([128, 1024], F32, tag="s", name="warm2")
            for i in range(24):
                nc.tensor.matmul(warm2_ps[:, 0:128], p2t, p2t,
                                 start=(i == 0), stop=(i == 23))
            for tn in range(12, NT):
                projB(tn)
    nc.compile()
    _cache["nc"] = nc
    return nc


def kernel(x, w_qkv, w_out, b_out, _trace=False):
    import ml_dtypes
    from concourse.bass_utils import run_bass_kernel_spmd

    x = np.asarray(x, dtype=np.float32)
    w_qkv = np.asarray(w_qkv, dtype=np.float32)
    w_out = np.asarray(w_out, dtype=np.float32)
    b_out = np.asarray(b_out, dtype=np.float32)

    cos2, sin2 = _rope_tables()
    p2t = _p2t()

    in_maps = []
    for c in range(N_CORES):
        b, g = divmod(c, G)
        cols = []
        for blk in range(2):                      # q block, k block
            base = blk * H * DH + g * G * DH
            cols.append(w_qkv[:, base:base + G * DH])
        wqk_c = np.ascontiguousarray(np.concatenate(cols, axis=1))  # [DIM, 512]
        wv_c = np.ascontiguousarray(
            w_qkv[:, 2 * H * DH + g * G * DH: 2 * H * DH + (g + 1) * G * DH])
        wout_c = np.ascontiguousarray(
            w_out[g * G * DH:(g + 1) * G * DH, :]).astype(ml_dtypes.bfloat16)
        in_maps.append({
            "xT": np.ascontiguousarray(x[b].T).astype(ml_dtypes.bfloat16),
            "wqk": wqk_c.astype(ml_dtypes.bfloat16),
            "wv": wv_c.astype(ml_dtypes.bfloat16),
            "wout": wout_c,
            "cos2": cos2.astype(ml_dtypes.bfloat16),
            "sin2": sin2.astype(ml_dtypes.bfloat16),
            "p2t": p2t.astype(ml_dtypes.bfloat16),
        })

    nc = _build()
    res = run_bass_kernel_spmd(nc, in_maps, core_ids=list(range(N_CORES)),
                               trace=_trace)
    out = np.empty((B, N, DIM), dtype=np.float32)
    for b in range(B):
        acc = res.results[G * b]["part"].astype(np.float32)
        for g in range(1, G):
            acc += res.results[G * b + g]["part"].astype(np.float32)
        out[b] = acc + b_out
    if _trace:
        kernel.last_results = res
    return out



# revision 73
# speedup vs baseline: 1.0112x; 1.0112x over previous
"""Trainium2 Bass kernel for nn_Attention_35021163332119.

Full multi-head attention: qkv = x @ w_qkv; RoPE(q, k); softmax(q k^T / sqrt(dh)) v;
out = heads @ w_out + b_out.  B=2, N=2048, DIM=1024, H=16, DH=64.

Sharding: 8 cores = (batch b in {0,1}) x (head-group g in {0..3} of 4 heads).
Each core computes its 4 heads end-to-end plus the partial output projection
for its head-group's rows of w_out; the host sums the 4 partials per batch
(bf16 partials, fp32 accumulation) and adds b_out.

Schedule: the kernel is PE-paced (~190 us of MATMUL busy at 2.4 GHz), so
the whole schedule keeps the Tensor engine dense and its HAM clock hot
(any idle re-throttles it to 1.2 GHz): deadline-ordered fused input DMAs
(a dma_start costs ~0.6 us of sequencer issue time, so per-k transfers
are fused into 3D-AP group loads), a per-half staged first block whose
scores start as soon as {wqk, xT half 0, trig} land, warm p2t spins
filling every DMA/rope-latency hole, and all remaining PE work (QKV
pair 1, RoPE, output projection) drip-fed as per-jj "filler" pieces.
The tail is latency-lean: the final block's normalization broadcasts via
GpSimd + partition-parallel reciprocal, trailing projection evacuations
run on the (idle-by-then) Scalar engine, and output DMAs issue from
three different sequencers.

On-core layout: x is host-transposed to xT [DIM, N]; q,k are produced
transposed ([dh, n], head pairs stacked on 128 partitions); v is produced
in natural [n, dh] layout with an extra ones column so the PV matmul (M=65)
also accumulates the softmax denominator in row 64.  RoPE's interleaved
pair-rotation is a 128x128 +/-1 permutation matmul on the PE plus DVE
multiplies against cos/sin tables (the cos one on GpSimd in the startup
phase).  PSUM->SBUF copies run on Scalar only while the exp stream has
not started; everything later uses DVE.
"""

import numpy as np

B, N, DIM, H, DH = 2, 2048, 1024, 16, 64
ROPE_BASE = 10000.0
SCALE = DH ** -0.5
N_CORES = 8
G = 4                 # heads per core
KT = DIM // 128       # contraction tiles
NT = N // 128         # sequence tiles

_cache = {}


def _rope_tables():
    inv_freq = (1.0 / (ROPE_BASE ** (np.arange(0, DH, 2, dtype=np.float32) / DH)))
    t = np.arange(N, dtype=np.float32)
    freqs = t[:, None] * inv_freq[None, :]          # [N, DH/2]
    freqs = np.repeat(freqs, 2, axis=-1)            # [N, DH] interleaved
    cosT = np.cos(freqs).T.astype(np.float32)       # [DH, N]
    sinT = np.sin(freqs).T.astype(np.float32)
    cos2 = np.concatenate([cosT, cosT], axis=0)     # [128, N] two heads stacked
    sin2 = np.concatenate([sinT, sinT], axis=0)
    return np.ascontiguousarray(cos2), np.ascontiguousarray(sin2)


def _p2t():
    # rot = P2 @ qT with P2 = blockdiag(P, P), P[2t, 2t+1] = -1, P[2t+1, 2t] = 1
    # matmul computes lhsT.T @ rhs, so pass P2.T
    p = np.zeros((DH, DH), dtype=np.float32)
    for t in range(DH // 2):
        p[2 * t, 2 * t + 1] = -1.0
        p[2 * t + 1, 2 * t] = 1.0
    p2 = np.zeros((128, 128), dtype=np.float32)
    p2[:DH, :DH] = p
    p2[DH:, DH:] = p
    return np.ascontiguousarray(p2.T)


def _build():
    if "nc" in _cache:
        return _cache["nc"]

    import concourse.mybir as mybir
    import concourse.tile as tile
    from concourse import bacc

    F32 = mybir.dt.float32
    F32R = mybir.dt.float32r
    BF16 = mybir.dt.bfloat16
    EXP = mybir.ActivationFunctionType.Exp

    nc = bacc.Bacc("TRN2", target_bir_lowering=False, debug=False)
    xT_d = nc.dram_tensor("xT", [DIM, N], BF16, kind="ExternalInput")
    wqk_d = nc.dram_tensor("wqk", [DIM, 4 * 128], BF16, kind="ExternalInput")
    wv_d = nc.dram_tensor("wv", [DIM, G * DH], BF16, kind="ExternalInput")
    wout_d = nc.dram_tensor("wout", [G * DH, DIM], BF16, kind="ExternalInput")
    cos_d = nc.dram_tensor("cos2", [128, N], BF16, kind="ExternalInput")
    sin_d = nc.dram_tensor("sin2", [128, N], BF16, kind="ExternalInput")
    p2t_d = nc.dram_tensor("p2t", [128, 128], BF16, kind="ExternalInput")
    part_d = nc.dram_tensor("part", [N, DIM], BF16, kind="ExternalOutput")

    with tile.TileContext(nc) as tc:
        with tc.tile_pool(name="persist", bufs=1) as persist, \
             tc.tile_pool(name="att", bufs=8) as att, \
             tc.tile_pool(name="norm_w", bufs=2) as norm_w, \
             tc.tile_pool(name="tailw", bufs=1) as tailw, \
             tc.tile_pool(name="outp", bufs=3) as outp, \
             tc.tile_pool(name="xph", bufs=1) as xph, \
             tc.tile_pool(name="rope_w", bufs=2) as rope_w, \
             tc.tile_pool(name="stash", bufs=4) as stash, \
             tc.tile_pool(name="ps", bufs=3, space="PSUM") as ps, \
             tc.tile_pool(name="pso", bufs=2, space="PSUM") as pso:

            # ---- persistent tiles ----
            # bf16 q/k: enables PE fast-weight-load on the scores matmuls
            # (halves the exposed LDWEIGHTS between row-group pairs) and 2x
            # DVE modes on the rope elementwise ops
            qk_sb = [persist.tile([128, N], BF16, tag=f"qk{m}", name=f"qk{m}")
                     for m in range(4)]          # q01T, q23T, k01T, k23T
            v_aug = persist.tile([128, NT, G, DH + 1], BF16, tag="vaug")
            wout_sb = [persist.tile([128, DIM], BF16, tag=f"wo{kk}", name=f"wo{kk}")
                       for kk in range(2)]
            outT = [persist.tile([128, N], BF16, tag=f"outT{p}", name=f"outT{p}")
                    for p in range(2)]

            # ---- phase-1 tiles (fused along the k-tile dim so one
            # dma_start loads a whole half-column group: a dma_start costs
            # ~0.6 us of sequencer issue time, so per-k transfers would make
            # the input phase issue-rate-bound) ----
            xT_all = xph.tile([128, KT, N], BF16, tag="xTall", name="xTall")
            wqk_all = xph.tile([128, KT, 4 * 128], BF16, tag="wqkall",
                               name="wqkall")
            wv_all = xph.tile([128, KT, G * DH], BF16, tag="wvall",
                              name="wvall")
            cos2 = xph.tile([128, N], BF16, tag="cos2")
            sin2 = xph.tile([128, N], BF16, tag="sin2")
            p2t = xph.tile([128, 128], BF16, tag="p2t")
            ones_bc = xph.tile([128, DH], F32, tag="ones_bc")
            warm = xph.tile([128, 8], F32, tag="warm")
            nc.vector.memset(ones_bc, 1.0)

            # preload the exp table set on the Scalar engine during DMA wait
            nc.vector.memset(warm, 0.0)
            nc.scalar.activation(warm, warm, EXP, scale=1.0)

            # ---- input DMAs, deadline-ordered, one fused start per group ----
            xT_src = xT_d.ap().rearrange("(t p) n -> p t n", p=128)
            wqk_src = wqk_d.ap().rearrange("(t p) m -> p t m", p=128)
            # p2t first: the PE warm-up matmuls below spin on it during the
            # DMA wait so the HAM clock gate opens before real work arrives
            nc.sync.dma_start(out=p2t, in_=p2t_d.ap())
            # q01/k01 weight columns + xT half 0 in fine-grained pieces so
            # the first chain's matmuls start as each piece lands
            nc.sync.dma_start(out=wqk_all[:, :, 256:384],
                              in_=wqk_src[:, :, 256:384])
            nc.sync.dma_start(out=wqk_all[:, :, 0:128],
                              in_=wqk_src[:, :, 0:128])
            nc.sync.dma_start(out=xT_all[:, 0:4, 0:512],
                              in_=xT_src[:, 0:4, 0:512])
            nc.sync.dma_start(out=xT_all[:, 4:KT, 0:512],
                              in_=xT_src[:, 4:KT, 0:512])
            nc.sync.dma_start(out=cos2[:, 0:1024], in_=cos_d.ap()[:, 0:1024])
            nc.sync.dma_start(out=sin2[:, 0:1024], in_=sin_d.ap()[:, 0:1024])
            nc.sync.dma_start(out=wqk_all[:, :, 128:256],
                              in_=wqk_src[:, :, 128:256])
            nc.sync.dma_start(out=wqk_all[:, :, 384:512],
                              in_=wqk_src[:, :, 384:512])
            nc.sync.dma_start(out=xT_all[:, :, 512:1024],
                              in_=xT_src[:, :, 512:1024])
            nc.sync.dma_start(
                out=wv_all,
                in_=wv_d.ap().rearrange("(t p) m -> p t m", p=128))
            nc.sync.dma_start(out=cos2[:, 1024:2048],
                              in_=cos_d.ap()[:, 1024:2048])
            nc.sync.dma_start(out=sin2[:, 1024:2048],
                              in_=sin_d.ap()[:, 1024:2048])
            nc.sync.dma_start(out=xT_all[:, :, 1024:1536],
                              in_=xT_src[:, :, 1024:1536])
            nc.sync.dma_start(out=xT_all[:, :, 1536:2048],
                              in_=xT_src[:, :, 1536:2048])
            for kk in range(2):
                nc.sync.dma_start(
                    out=wout_sb[kk],
                    in_=wout_d.ap().rearrange("(t p) m -> t p m", p=128)[kk])
            nc.vector.memset(v_aug[:, :, :, DH:DH + 1], 1.0)

            # ---- building blocks ----
            def qk_half(m, c2, half, slot, pair_ps, on_act):
                """8 accumulating matmuls for the 512-wide half (m, c2,
                half), written into `slot` (0/1) of a shared [128,1024] PSUM
                tile (two independent bank-aligned accumulation groups),
                then copied out to qk_sb on Scalar (pre-exp-stream) or DVE."""
                csl = slice(c2 * 1024 + half * 512, c2 * 1024 + (half + 1) * 512)
                sl = slice(slot * 512, (slot + 1) * 512)
                for k in range(KT):
                    nc.tensor.matmul(
                        pair_ps[:, sl],
                        wqk_all[:, k, m * 128:(m + 1) * 128],
                        xT_all[:, k, csl],
                        start=(k == 0), stop=(k == KT - 1))
                if on_act:
                    nc.scalar.copy(qk_sb[m][:, csl], pair_ps[:, sl])
                else:
                    nc.vector.tensor_copy(qk_sb[m][:, csl], pair_ps[:, sl])

            def rope_m_half(m, c2, half, cos_on_gpsimd=False):
                csl = slice(c2 * 1024 + half * 512, c2 * 1024 + (half + 1) * 512)
                rot_ps = ps.tile([128, 1024], F32, tag="s", name="mm_rot")
                nc.tensor.matmul(rot_ps[:, 0:512], p2t, qk_sb[m][:, csl],
                                 start=True, stop=True)
                tmp = rope_w.tile([128, 1024], BF16, tag="ropetmp")
                nc.vector.tensor_mul(tmp[:, 0:512], rot_ps[:, 0:512],
                                     sin2[:, csl])
                # the cos multiply touches only SBUF, so in the (gpsimd-idle)
                # startup phase it can run in parallel with the sin multiply
                cos_eng = nc.gpsimd if cos_on_gpsimd else nc.vector
                cos_eng.tensor_mul(qk_sb[m][:, csl], qk_sb[m][:, csl],
                                   cos2[:, csl])
                nc.vector.tensor_add(qk_sb[m][:, csl], qk_sb[m][:, csl],
                                     tmp[:, 0:512])

            def v_pair(t2):
                """v for sequence tiles 2*t2, 2*t2+1 in one PSUM tile."""
                mm_ps = ps.tile([128, 1024], F32, tag="s", name="mm_v")
                for sub in range(2):
                    tn = 2 * t2 + sub
                    for k in range(KT):
                        nc.tensor.matmul(
                            mm_ps[:, sub * 256:(sub + 1) * 256],
                            xT_all[:, k, tn * 128:(tn + 1) * 128],
                            wv_all[:, k, :],
                            start=(k == 0), stop=(k == KT - 1))
                nc.vector.tensor_copy(
                    v_aug[:, 2 * t2:2 * t2 + 2, :, 0:DH],
                    mm_ps[:, 0:512].rearrange("p (t h d) -> p t h d", t=2, h=G))

            # ---- attention machinery ----
            def att_begin():
                return {"o_ps": [pso.tile([DH + 1, 512], F32, tag="o",
                                          name=f"o{hh}") for hh in range(2)],
                        "pends": []}

            def emit_pv(p, st, jj, exps):
                for hh in range(2):
                    for half in range(2):
                        j = 2 * jj + half
                        nc.tensor.matmul(
                            st["o_ps"][hh],
                            v_aug[:, j, 2 * p + hh, :],
                            exps[hh][:, half * 512:(half + 1) * 512],
                            start=(j == 0), stop=(j == NT - 1))

            def att_jj(p, iq, jj, st, fill=None):
                # NOTE: fill must stay AFTER the exps — emitting it first
                # was measured ~3 us slower (the in-order PE stalls on the
                # fill's input deps before reaching ready scores work)
                qT, kTt = qk_sb[p], qk_sb[2 + p]
                isl = slice(iq * 512, (iq + 1) * 512)
                s_ps = [ps.tile([128, 1024], F32, tag="s", name=f"s{hh}")
                        for hh in range(2)]
                for half in range(2):
                    j = 2 * jj + half
                    jsl = slice(j * 128, (j + 1) * 128)
                    for hh in range(2):
                        hsl = slice(hh * DH, (hh + 1) * DH)
                        nc.tensor.matmul(
                            s_ps[hh][:, half * 512:(half + 1) * 512],
                            kTt[hsl, jsl], qT[hsl, isl],
                            start=True, stop=True)
                exps = []
                for hh in range(2):
                    expT = att.tile([128, 1024], BF16, tag="exp")
                    nc.scalar.activation(expT, s_ps[hh], EXP, scale=SCALE)
                    exps.append(expT)
                if fill is not None:
                    fill()
                # depth-2 PV pipeline: a PV group is emitted two jj after its
                # exps, so it never waits on the second head's activation
                st["pends"].append((jj, exps))
                if len(st["pends"]) > 2:
                    j0, e0 = st["pends"].pop(0)
                    emit_pv(p, st, j0, e0)

            def att_end(p, iq, st, fast_tail=False):
                for (j0, e0) in st["pends"]:
                    emit_pv(p, st, j0, e0)
                st["pends"] = []
                isl = slice(iq * 512, (iq + 1) * 512)
                # hh=1 first: its chain ends in an SBUF->SBUF DMA hop into
                # outT, which then overlaps hh=0's direct DVE write -- the
                # projection consumers wait on whichever finishes last
                for hh in (1, 0):
                    o_sb = norm_w.tile([DH + 1, 512], F32, tag=f"osb{hh}",
                                       name=f"osb{hh}")
                    nc.vector.tensor_copy(o_sb, st["o_ps"][hh])
                    if fast_tail:
                        # latency-lean: reciprocal in place on partition 64,
                        # broadcast via a K=1 matmul — no DMA hop, no gpsimd
                        nc.vector.reciprocal_approx_fast(
                            o_sb[DH:DH + 1, :], o_sb[DH:DH + 1, :])
                        bc_ps = ps.tile([128, 1024], F32, tag="s", name="bc")
                        nc.tensor.matmul(bc_ps[0:DH, 0:512],
                                         ones_bc[DH:DH + 1, 0:DH],
                                         o_sb[DH:DH + 1, :],
                                         start=True, stop=True)
                        bc = bc_ps[0:DH, 0:512]
                    else:
                        recip0 = norm_w.tile([1, 512], F32, tag=f"r0{hh}",
                                             name=f"r0{hh}")
                        nc.sync.dma_start(out=recip0, in_=o_sb[DH:DH + 1, :])
                        nc.vector.reciprocal_approx_fast(recip0, recip0)
                        bc = norm_w.tile([DH, 512], F32, tag=f"bc{hh}",
                                         name=f"bc{hh}")
                        nc.gpsimd.partition_broadcast(bc, recip0)
                    if hh == 0:
                        nc.vector.tensor_mul(outT[p][0:DH, isl],
                                             o_sb[0:DH, :], bc)
                    else:
                        tmpb = norm_w.tile([DH, 512], BF16, tag="tmpb")
                        nc.vector.tensor_mul(tmpb, o_sb[0:DH, :], bc)
                        nc.sync.dma_start(out=outT[p][DH:2 * DH, isl],
                                          in_=tmpb)

            def att_norm_tail_recips(st):
                """Phase 1 of the final block's latency-lean norm: broadcast
                each head's denominator row straight out of PSUM on the
                (idle) GpSimd engine, then take the reciprocal partition-
                parallel on DVE.  No DMA hop, no PE broadcast matmul."""
                recs = []
                for hh in (1, 0):
                    dsb = tailw.tile([1, 512], F32, tag=f"dt{hh}",
                                     name=f"dt{hh}")
                    nc.vector.tensor_copy(dsb, st["o_ps"][hh][DH:DH + 1, :])
                    bc = tailw.tile([DH, 512], F32, tag=f"bt{hh}",
                                    name=f"bt{hh}")
                    nc.gpsimd.partition_broadcast(bc, dsb)
                    nc.vector.reciprocal_approx_fast(bc, bc)
                    recs.append((hh, bc))
                return recs

            def att_norm_tail_muls(p, iq, st, recs):
                """Phase 2: scale the raw PSUM outputs by the broadcast
                reciprocals and write both head halves straight to outT."""
                isl = slice(iq * 512, (iq + 1) * 512)
                for hh, bc in recs:
                    nc.vector.tensor_mul(
                        outT[p][hh * DH:(hh + 1) * DH, isl],
                        st["o_ps"][hh][0:DH, :], bc)

            tail_mode = {"on": False}

            def proj_half(tn, c2, holder):
                """One 512-wide half of projection tile tn; copy+DMA on the
                second half.  In tail mode the PSUM evacuation runs on the
                (by then idle) Scalar engine so DVE is free for the final
                norm muls and adds."""
                if c2 == 0:
                    holder.clear()
                    holder.append(ps.tile([128, 1024], F32, tag="s",
                                          name="f_ps"))
                f_ps = holder[0]
                nsl = slice(tn * 128, (tn + 1) * 128)
                c2sl = slice(c2 * 512, (c2 + 1) * 512)
                for kk in range(2):
                    nc.tensor.matmul(
                        f_ps[:, c2sl],
                        outT[kk][:, nsl], wout_sb[kk][:, c2sl],
                        start=(kk == 0), stop=(kk == 1))
                if c2 == 1:
                    out_sb = outp.tile([128, DIM], BF16, tag="osb")
                    if tail_mode["on"]:
                        nc.scalar.copy(out_sb, f_ps)
                    else:
                        nc.vector.tensor_copy(out_sb, f_ps)
                    pdram = part_d.ap().rearrange("(t p) m -> t p m", p=128)[tn]
                    for phh in range(2):
                        psl2 = slice(phh * 64, (phh + 1) * 64)
                        nc.sync.dma_start(out=pdram[psl2, :],
                                          in_=out_sb[psl2, :])

            def proj_tile(tn):
                holder = []
                for c2 in range(2):
                    proj_half(tn, c2, holder)

            # ---- emission ----
            # Scheduling principle: the Tensor engine must stay ~100% dense
            # (its HAM clock gate re-throttles to 1.2 GHz if it idles and
            # never re-warms without a ~3.4us sustained-busy window), while
            # the Scalar engine's exp stream should pace the kernel.  So
            # every attention jj carries a small "filler" slice of the
            # non-attention PE work, slightly oversubscribing the PE.

            # PE warm-up: tiny matmuls on p2t spin during the input DMA
            # wait so the HAM opens the clock gate before the QKV work lands.
            # warm_fill() is also sprinkled into otherwise-idle early slots:
            # any PE idle resets the clock ramp back to 1.2 GHz, which would
            # make the whole DMA-gated prefix run at half speed.
            warm_ps = ps.tile([128, 1024], F32, tag="s", name="warm_ps")

            def warm_fill(n):
                for i in range(n):
                    nc.tensor.matmul(warm_ps[:, 0:128], p2t, p2t,
                                     start=(i == 0), stop=(i == n - 1))

            warm_fill(20)

            # First block (pair 0, i-quarter 0), staged per 512-wide half so
            # the first scores fire as soon as {wqk, xT half 0, trig chunk 0}
            # land (~17 us) instead of waiting for the full chunk-0 QKV.
            # Block (0,0) only needs q01 c0h0 (i 0:512) plus k01 halves
            # progressively (jj0-1: c0h0, jj2-3: c0h1, jj4-5: c1h0,
            # jj6-7: c1h1); each k01 half-chain is paired in its PSUM tile
            # with a pair-1 half-chain (q23/k23/q01-next) whose rope is
            # deferred, and the v chains ride as per-jj fills.
            st = att_begin()
            pA = ps.tile([128, 1024], F32, tag="s", name="chA")
            qk_half(2, 0, 0, 0, pA, True)
            qk_half(0, 0, 0, 1, pA, True)
            rope_m_half(2, 0, 0, cos_on_gpsimd=True)
            rope_m_half(0, 0, 0, cos_on_gpsimd=True)
            warm_fill(20)      # spin out the rope DVE latency at full clock
            att_jj(0, 0, 0, st, fill=(lambda: warm_fill(10)))
            att_jj(0, 0, 1, st, fill=(lambda: warm_fill(10)))
            pB2 = ps.tile([128, 1024], F32, tag="s", name="chB")
            qk_half(2, 0, 1, 0, pB2, False)
            qk_half(1, 0, 0, 1, pB2, False)
            rope_m_half(2, 0, 1, cos_on_gpsimd=True)
            warm_fill(8)
            att_jj(0, 0, 2, st, fill=(lambda: v_pair(0)))
            att_jj(0, 0, 3, st, fill=(lambda: v_pair(1)))
            pC = ps.tile([128, 1024], F32, tag="s", name="chC")
            qk_half(2, 1, 0, 0, pC, False)
            qk_half(3, 1, 0, 1, pC, False)
            rope_m_half(2, 1, 0, cos_on_gpsimd=True)
            warm_fill(8)
            att_jj(0, 0, 4, st, fill=(lambda: (v_pair(2), v_pair(3))))
            att_jj(0, 0, 5, st, fill=(lambda: v_pair(4)))
            pD = ps.tile([128, 1024], F32, tag="s", name="chD")
            qk_half(2, 1, 1, 0, pD, False)
            qk_half(0, 0, 1, 1, pD, False)
            rope_m_half(2, 1, 1)
            warm_fill(8)
            att_jj(0, 0, 6, st, fill=(lambda: (v_pair(5), v_pair(6))))
            # q01 c0h1 (computed in chain D) feeds block (0,1)'s very first
            # scores: rope it inside jj7 so its DVE work clears during this
            # block's tail instead of stalling the next block's start
            att_jj(0, 0, 7, st, fill=(lambda: (v_pair(7),
                                               rope_m_half(0, 0, 1))))
            att_end(0, 0, st)

            # filler pieces: remaining QKV half-chains (kpair granularity)
            # + their ropes, in deadline order; consumed up to 2 per jj
            # across blocks (0,1)-(0,3) and the spare slots of (1,0)
            fillers = []

            def qk_kpair_h(m, c2, half, kp, holder):
                csl = slice(c2 * 1024 + half * 512,
                            c2 * 1024 + (half + 1) * 512)
                if kp == 0:
                    holder.clear()
                    holder.append(ps.tile([128, 1024], F32, tag="s",
                                          name="qk1"))
                mm_ps = holder[0]
                for k in range(2 * kp, 2 * kp + 2):
                    nc.tensor.matmul(
                        mm_ps[:, 0:512],
                        wqk_all[:, k, m * 128:(m + 1) * 128],
                        xT_all[:, k, csl],
                        start=(k == 0), stop=(k == KT - 1))
                if kp == 3:
                    nc.vector.tensor_copy(qk_sb[m][:, csl], mm_ps[:, 0:512])

            def add_chain(m, c2, half):
                h2 = []
                for kp in range(4):
                    fillers.append(
                        lambda m=m, c2=c2, half=half, kp=kp, h2=h2:
                        qk_kpair_h(m, c2, half, kp, h2))
                fillers.append(
                    lambda m=m, c2=c2, half=half: rope_m_half(m, c2, half))

            add_chain(0, 1, 0)                            # q01 c1h0: (0,2)
            add_chain(3, 0, 0)                            # k23 c0h0: (1,0)
            add_chain(0, 1, 1)                            # q01 c1h1: (0,3)
            add_chain(3, 0, 1)                            # k23 c0h1: (1,0)
            add_chain(3, 1, 1)                            # k23 c1h1: (1,0)
            fillers.append(lambda: rope_m_half(3, 1, 0))  # chain in (0,0)
            fillers.append(lambda: rope_m_half(1, 0, 0))  # chain in (0,0)
            add_chain(1, 0, 1)                            # q23 c0h1: (1,1)
            add_chain(1, 1, 0)                            # q23 c1h0: (1,2)
            add_chain(1, 1, 1)                            # q23 c1h1: (1,3)

            for (p, iq) in [(0, 1), (0, 2), (0, 3)]:
                st = att_begin()
                for jj in range(NT // 2):
                    fill = None
                    if jj >= 1 and fillers:
                        pieces = [fillers.pop(0)]
                        if fillers:
                            pieces.append(fillers.pop(0))
                        fill = (lambda ps_=pieces: [f() for f in ps_])
                    att_jj(p, iq, jj, st, fill)
                att_end(p, iq, st)

            # two-phase projection for the trailing tiles 12-15: the
            # outT[0]-side accumulation only needs pair-0 results, so it
            # runs as fills inside block (1, 0); after the last block only
            # the outT[1]-side matmuls + add + DMA remain.
            pstash = {}

            def projA(tn):
                nsl = slice(tn * 128, (tn + 1) * 128)
                f_ps = ps.tile([128, 1024], F32, tag="s", name="pA")
                for c2 in range(2):
                    c2sl = slice(c2 * 512, (c2 + 1) * 512)
                    nc.tensor.matmul(f_ps[:, c2sl], outT[0][:, nsl],
                                     wout_sb[0][:, c2sl],
                                     start=True, stop=True)
                sb = stash.tile([128, DIM], F32, tag="pst", name=f"pst{tn}")
                nc.vector.tensor_copy(sb, f_ps)
                pstash[tn] = sb

            def projB(tn):
                """Tail projection: DVE add folds in the stashed outT0-side
                partial; the 4 row-contiguous 32 KB output chunks issue from
                Scalar+Sync (GpSimd must not issue the final transfers — its
                exit drain waits on its own queue and delays the finale)."""
                nsl = slice(tn * 128, (tn + 1) * 128)
                f_ps = ps.tile([128, 1024], F32, tag="s", name="pB")
                for c2 in range(2):
                    c2sl = slice(c2 * 512, (c2 + 1) * 512)
                    nc.tensor.matmul(f_ps[:, c2sl], outT[1][:, nsl],
                                     wout_sb[1][:, c2sl],
                                     start=True, stop=True)
                out_sb = outp.tile([128, DIM], BF16, tag="osb")
                nc.vector.tensor_add(out_sb, f_ps, pstash[tn])
                pdram = part_d.ap().rearrange("(t p) m -> t p m", p=128)[tn]
                engs = (nc.scalar, nc.sync, nc.scalar, nc.sync)
                for phh, eng in enumerate(engs):
                    psl2 = slice(phh * 32, (phh + 1) * 32)
                    eng.dma_start(out=pdram[psl2, :], in_=out_sb[psl2, :])

            st = att_begin()
            for jj in range(NT // 2):
                fill = None
                if 2 <= jj < 6:
                    fill = (lambda tn=10 + jj: projA(tn))
                elif jj >= 1 and fillers:
                    pieces = [fillers.pop(0)]
                    if fillers:
                        pieces.append(fillers.pop(0))
                    fill = (lambda ps_=pieces: [f() for f in ps_])
                att_jj(1, 0, jj, st, fill)
            att_end(1, 0, st)
            # anything q23 still owes must precede block (1,1)'s scores
            while fillers:
                fillers.pop(0)()

            # projection fillers for tiles 0-11: tile tn needs outT i-chunk
            # tn//4, finished at att_end(1, tn//4); two halves per tile, one
            # per jj starting two jj into the following block
            projq = []
            for (bi, (p, iq)) in enumerate([(1, 1), (1, 2), (1, 3)]):
                for tn in range(4 * bi, 4 * bi + 4):
                    h3 = []
                    for c2 in range(2):
                        projq.append(
                            (bi, lambda tn=tn, c2=c2, h3=h3:
                             proj_half(tn, c2, h3)))
                st = att_begin()
                for jj in range(NT // 2):
                    fill = None
                    if jj >= 2 and projq and projq[0][0] <= bi:
                        fill = projq.pop(0)[1]
                        if jj >= 4 and projq and projq[0][0] < bi:
                            f1, f2 = fill, projq.pop(0)[1]
                            fill = (lambda a=f1, b=f2: (a(), b()))
                    att_jj(p, iq, jj, st, fill)
                if (p, iq) != (1, 3):
                    att_end(p, iq, st)
                else:
                    # final block: drain PV only, then put all remaining
                    # independent PE work BEFORE the norm chain so the
                    # in-order PE queue isn't blocked behind the DVE recips
                    for (j0, e0) in st["pends"]:
                        emit_pv(p, st, j0, e0)
                    st["pends"] = []
            recs = att_norm_tail_recips(st)
            # muls BEFORE the projq drain so they aren't queued behind the
            # leftover projection evacuations on DVE
            att_norm_tail_muls(1, 3, st, recs)
            tail_mode["on"] = True
            for (_, fn) in projq:
                fn()
            # keep the PE clock warm through the final norm chain so the
            # trailing projection matmuls run at 2.4 GHz
            warm2_ps = ps.tile([128, 1024], F32, tag="s", name="warm2")
            for i in range(24):
                nc.tensor.matmul(warm2_ps[:, 0:128], p2t, p2t,
                                 start=(i == 0), stop=(i == 23))
            for tn in range(12, NT):
                projB(tn)
    nc.compile()
    _cache["nc"] = nc
    return nc


def kernel(x, w_qkv, w_out, b_out, _trace=False):
    import ml_dtypes
    from concourse.bass_utils import run_bass_kernel_spmd

    x = np.asarray(x, dtype=np.float32)
    w_qkv = np.asarray(w_qkv, dtype=np.float32)
    w_out = np.asarray(w_out, dtype=np.float32)
    b_out = np.asarray(b_out, dtype=np.float32)

    cos2, sin2 = _rope_tables()
    p2t = _p2t()

    in_maps = []
    for c in range(N_CORES):
        b, g = divmod(c, G)
        cols = []
        for blk in range(2):                      # q block, k block
            base = blk * H * DH + g * G * DH
            cols.append(w_qkv[:, base:base + G * DH])
        wqk_c = np.ascontiguousarray(np.concatenate(cols, axis=1))  # [DIM, 512]
        wv_c = np.ascontiguousarray(
            w_qkv[:, 2 * H * DH + g * G * DH: 2 * H * DH + (g + 1) * G * DH])
        wout_c = np.ascontiguousarray(
            w_out[g * G * DH:(g + 1) * G * DH, :]).astype(ml_dtypes.bfloat16)
        in_maps.append({
            "xT": np.ascontiguousarray(x[b].T).astype(ml_dtypes.bfloat16),
            "wqk": wqk_c.astype(ml_dtypes.bfloat16),
            "wv": wv_c.astype(ml_dtypes.bfloat16),
            "wout": wout_c,
            "cos2": cos2.astype(ml_dtypes.bfloat16),
            "sin2": sin2.astype(ml_dtypes.bfloat16),
            "p2t": p2t.astype(ml_dtypes.bfloat16),
        })

    nc = _build()
    res = run_bass_kernel_spmd(nc, in_maps, core_ids=list(range(N_CORES)),
                               trace=_trace)
    out = np.empty((B, N, DIM), dtype=np.float32)
    for b in range(B):
        acc = res.results[G * b]["part"].astype(np.float32)
        for g in range(1, G):
            acc += res.results[G * b + g]["part"].astype(np.float32)
        out[b] = acc + b_out
    if _trace:
        kernel.last_results = res
    return out

